# revision 18
# baseline (speedup 1.0000x reference)
"""Causal self-attention on 8 TRN2 NeuronCores.  Baseline schedule +
replicated-denominator normalize (ones-block in V' fills PV rows 64:127
with the softmax denominator, so normalize is a PSUM->SBUF copy + approx
reciprocal + multiply; no GpSimd partition_broadcast on the critical path).

Sharding: core c handles batch b = c//4 and heads [3g, 3g+1, 3g+2] with
g = c%4 (data parallel on B x tensor parallel on heads).  Each core computes
its heads' attention output projected through its slice of w_proj; the host
sums the 4 partial [T, C] outputs per batch and adds b_proj.
"""

import numpy as np

N_CORES = 8
B = 2
T = 4096
C = 768
NH = 12
HD = 64
HPC = 3            # heads per core
TCH = 512          # t / q chunk
KCH = 128          # k chunk
CPART = 128

_cache = {}


def _ensure_axon_hooks_module():
    import sys
    import types
    try:
        import antenv.axon_hooks  # noqa: F401
        return
    except Exception:
        pass
    m = types.ModuleType("antenv.axon_hooks")
    m._hook = None

    def _set(h):
        m._hook = h

    def _get():
        return m._hook

    m.set_axon_ntff_profile_hook = _set
    m.get_axon_ntff_profile_hook = _get
    sys.modules["antenv.axon_hooks"] = m


def build_program(t=T):
    """Build the single-core SPMD bass program (same program on all cores,
    per-core data). Returns the un-finalized Bacc."""
    import concourse.mybir as mybir
    import concourse.tile as tile
    from concourse import bacc
    from concourse.bass import ds, ts

    f32 = mybir.dt.float32
    bf16 = mybir.dt.bfloat16
    AF = mybir.ActivationFunctionType

    nt = t // TCH          # number of t/q chunks
    spk = TCH // KCH       # k-chunks per t-chunk (4)
    cc_n = C // CPART      # 6 contraction chunks

    nc = bacc.Bacc("TRN2", target_bir_lowering=False)

    PK_W = 6 * 128 * 3 + 6 * 192 + C + C + 128 + 384
    xT = nc.dram_tensor("xT", [128, (t // TCH) * (C // CPART) * TCH], bf16,
                        kind="ExternalInput")
    wpk_d = nc.dram_tensor("wpk", [128, PK_W], bf16, kind="ExternalInput")
    bpk_d = nc.dram_tensor("bpk", [128, 3], f32, kind="ExternalInput")
    out_d = nc.dram_tensor("out", [t, C], bf16, kind="ExternalOutput")

    with tile.TileContext(nc) as tc_:
        with (
            tc_.tile_pool(name="consts", bufs=1) as consts,
            tc_.tile_pool(name="big", bufs=1) as big,
            tc_.tile_pool(name="xin", bufs=3) as xin,
            tc_.tile_pool(name="ptp", bufs=10) as ptp,
            tc_.tile_pool(name="wkp", bufs=4) as wkp,
            tc_.tile_pool(name="sps", bufs=2, space="PSUM") as sps,
            tc_.tile_pool(name="ops", bufs=4, space="PSUM") as ops,
        ):
            wpk = consts.tile([128, PK_W], bf16)
            bpk = consts.tile([128, 3], f32)

            def emit_weight_loads_head():
                nc.sync.dma_start(wpk[:, 0:768], wpk_d[:, 0:768])

            def emit_bias_load():
                nc.sync.dma_start(bpk[:], bpk_d[:, :])

            def emit_weight_loads_rest():
                nc.sync.dma_start(wpk[:, 768:2304], wpk_d[:, 768:2304])
                nc.sync.dma_start(wpk[:, 2304:3474], wpk_d[:, 2304:3474])
                nc.sync.dma_start(wpk[:, 3474:PK_W], wpk_d[:, 3474:PK_W])

            warm_src = consts.tile([128, 512], bf16)
            nc.vector.memset(warm_src[:], 0.0)

            def emit_warm(n):
                wps = sps.tile([128, 512], f32, tag="S", name="warm")
                for _ in range(n):
                    nc.tensor.matmul(wps[:], warm_src[:, 0:128],
                                     warm_src[:], start=True, stop=True)

            def seg(off, w):
                ap = wpk[:, off:off + w]
                return ap, off + w

            _o = 0
            wq01_f, _o = seg(_o, 6 * 128)
            wk01_f, _o = seg(_o, 6 * 128)
            wqk2_f, _o = seg(_o, 6 * 128)
            wv_f, _o = seg(_o, 6 * 192)
            wpA_sb, _o = seg(_o, C)
            wpB_full, _o = seg(_o, C)
            tri_sb, _o = seg(_o, 128)
            misc_f, _o = seg(_o, 384)
            wq01_sb = wq01_f.rearrange("p (c m) -> p c m", c=cc_n)
            wk01_sb = wk01_f.rearrange("p (c m) -> p c m", c=cc_n)
            wqk2_sb = wqk2_f.rearrange("p (c m) -> p c m", c=cc_n)
            wv_sb = wv_f.rearrange("p (c m) -> p c m", c=cc_n)
            wpB_lo = wpB_full[0:64, :]
            wpB_hi = wpB_full[64:128, :]
            del misc_f  # reserved pack space, currently unused
            bq01_sb = bpk[:, 0:1]
            bk01_sb = bpk[:, 1:2]
            bqk2_sb = bpk[:, 2:3]

            # ---- persistent activations ----
            Q01 = big.tile([128, t], bf16)
            K01 = big.tile([128, t], bf16)
            Q2 = big.tile([128, t], bf16)
            K2 = big.tile([128, t], bf16)
            Vp = big.tile([128, t // KCH, 384], bf16)
            Vp_h = Vp[:, :, :].rearrange("p n (h x) -> p n h x", h=3)
            yTa = big.tile([128, t], bf16)
            yT2 = big.tile([128, t], bf16)

            xT_r = xT[:, :].rearrange("p (nt c m) -> p nt c m", nt=nt,
                                      c=cc_n)

            def qkv_ops(tci):
                state = {}
                ops_l = []

                def dma_cast():
                    xtb = xin.tile([128, cc_n, TCH], bf16, tag="xtb",
                                   name="xtb")
                    if tci == 0:
                        for cc in range(0, cc_n, 2):
                            nc.sync.dma_start(xtb[:, cc:cc + 2, :],
                                              xT_r[:, tci, cc:cc + 2, :])
                    else:
                        nc.sync.dma_start(xtb[:], xT_r[:, tci, :, :])
                    state["xtb"] = xtb
                ops_l.append(dma_cast)

                def qk_set(wsb, bsb, dst):
                    xtb = state["xtb"]
                    qkps = sps.tile([128, TCH], f32, tag="S", name="qkps")
                    for cc in range(cc_n):
                        nc.tensor.matmul(
                            qkps[:], wsb[:, cc, :], xtb[:, cc, :],
                            start=(cc == 0), stop=(cc == cc_n - 1))
                    if dst is None:
                        nc.vector.tensor_scalar_add(
                            Q2[0:64, ts(tci, TCH)], qkps[0:64, :],
                            bsb[0:64, :])
                        nc.vector.tensor_scalar_add(
                            K2[64:128, ts(tci, TCH)], qkps[64:128, :],
                            bsb[64:128, :])
                        nc.sync.dma_start(Q2[64:128, ts(tci, TCH)],
                                          Q2[0:64, ts(tci, TCH)])
                        nc.sync.dma_start(K2[0:64, ts(tci, TCH)],
                                          K2[64:128, ts(tci, TCH)])
                    else:
                        nc.vector.tensor_scalar_add(
                            dst[:, ts(tci, TCH)], qkps[:], bsb[:])

                for wsb, bsb, dst in (
                    (wq01_sb, bq01_sb, Q01),
                    (wk01_sb, bk01_sb, K01),
                    (wqk2_sb, bqk2_sb, None),
                ):
                    ops_l.append(
                        lambda w=wsb, b=bsb, d=dst: qk_set(w, b, d))

                def v_set(st):
                    xtb = state["xtb"]
                    tt = tci * spk + st
                    vps = ops.tile([128, 192], f32, tag="oT", name="vps")
                    for cc in range(cc_n):
                        nc.tensor.matmul(
                            vps[:], xtb[:, cc, ts(st, 128)], wv_sb[:, cc, :],
                            start=(cc == 0), stop=(cc == cc_n - 1))
                    nc.vector.tensor_copy(
                        Vp_h[:, tt, :, 0:64],
                        vps[:].rearrange("p (h x) -> p h x", h=3))
                    nc.vector.memset(Vp_h[:, tt, :, 64:128], 1.0)

                for st in range(spk):
                    ops_l.append(lambda s=st: v_set(s))
                return ops_l

            def proj_ops(tci, tail=False):
                def do_tile(tt):
                    po1 = ops.tile([128, 512], f32, tag="oT", name="po1")
                    po2 = ops.tile([128, 256], f32, tag="oT", name="po2")
                    for po, cs, cw in ((po1, 0, 512), (po2, 512, 256)):
                        nc.tensor.matmul(po[:], yTa[:, ts(tt, 128)],
                                         wpA_sb[:, ds(cs, cw)],
                                         start=True, stop=False)
                        nc.tensor.matmul(po[:], yT2[0:64, ts(tt, 128)],
                                         wpB_lo[:, ds(cs, cw)],
                                         start=False, stop=True)
                    pout = xin.tile([128, C], bf16, tag="pout", name="pout")
                    if tail:
                        nc.scalar.activation(pout[:, 0:512], po1[:], AF.Copy)
                    else:
                        nc.vector.tensor_copy(pout[:, 0:512], po1[:])
                    nc.vector.tensor_copy(pout[:, 512:768], po2[:])
                    nc.sync.dma_start(out_d[ds(tt * 128, 64), :],
                                      pout[0:64, :])
                    nc.sync.dma_start(out_d[ds(tt * 128 + 64, 64), :],
                                      pout[64:128, :])

                return [lambda x=(tci * spk + s): do_tile(x)
                        for s in range(spk)]

            def normalize(oT, h, qc):
                den = wkp.tile([64, TCH], f32, tag="den", name="den")
                nc.vector.tensor_copy(den[:], oT[64:128, :])
                rb = wkp.tile([64, TCH], f32, tag="rb", name="rb")
                nc.vector.reciprocal_approx_fast(out=rb[:], in_=den[:])
                if h == 0:
                    nc.vector.tensor_mul(yTa[0:64, ts(qc, TCH)], oT[0:64, :],
                                         rb[:])
                elif h == 2:
                    nc.vector.tensor_mul(yT2[0:64, ts(qc, TCH)], oT[0:64, :],
                                         rb[:])
                else:
                    y1t = wkp.tile([64, TCH], bf16, tag="y1t", name="y1t")
                    nc.vector.tensor_mul(y1t[:], oT[0:64, :], rb[:])
                    nc.sync.dma_start(yTa[64:128, ts(qc, TCH)], y1t[:])

            pend = {}

            def emit_s01(qc, kc):
                nkc = (qc + 1) * spk
                q0 = qc * TCH
                m = kc - qc * spk
                lo = max(0, 128 * m)
                S = sps.tile([128, 1024], f32, tag="S", name="S01")
                nc.tensor.matmul(
                    S[:, lo:TCH],
                    K01[0:64, ts(kc, KCH)], Q01[0:64, ds(q0 + lo,
                                                         TCH - lo)],
                    start=True, stop=True, tile_position=(0, 0))
                nc.tensor.matmul(
                    S[:, TCH + lo:1024],
                    K01[64:128, ts(kc, KCH)], Q01[64:128, ds(q0 + lo,
                                                             TCH - lo)],
                    start=True, stop=True, tile_position=(64, 0))
                pend[(qc, "p1", kc)] = S

            def emit_s2(qc, kp):
                q0 = qc * TCH
                kc0, kc1 = 2 * kp, 2 * kp + 1
                lo0 = max(0, 128 * (kc0 - qc * spk))
                S2 = sps.tile([128, 1024], f32, tag="S", name="S2")
                nc.tensor.matmul(
                    S2[:, lo0:TCH],
                    K2[0:64, ts(kc0, KCH)], Q2[0:64, ds(q0 + lo0,
                                                        TCH - lo0)],
                    start=True, stop=True, tile_position=(0, 0))
                nc.tensor.matmul(
                    S2[:, TCH + lo0:1024],
                    K2[64:128, ts(kc1, KCH)], Q2[64:128, ds(q0 + lo0,
                                                            TCH - lo0)],
                    start=True, stop=True, tile_position=(64, 0))
                pend[(qc, "p2", kp)] = S2

            def attention(qc, fillers):
                nkc = (qc + 1) * spk
                npair = nkc // 2

                def lo_of(kc):
                    m = kc - qc * spk
                    return max(0, 128 * m), m

                # ---- pass 1: heads 0,1 row-tiled ----
                oT0 = ops.tile([128, TCH], f32, tag="oT", name="oT0")
                oT1 = ops.tile([128, TCH], f32, tag="oT", name="oT1")
                for idx in (0, 1):
                    if (qc, "p1", idx) not in pend and idx < nkc:
                        emit_s01(qc, idx)
                for kc in range(nkc):
                    if fillers:
                        fillers.pop(0)()
                    lo, m = lo_of(kc)
                    S = pend.pop((qc, "p1", kc))
                    PT = ptp.tile([128, 1024], bf16, tag="PT", name="PT")
                    if lo == 0:
                        nc.scalar.activation(PT[:], S[:], AF.Exp, scale=0.125)
                    else:
                        s_v = S[:].rearrange("p (h q) -> p h q", h=2)[
                            :, :, lo:TCH]
                        p_v = PT[:].rearrange("p (h q) -> p h q", h=2)[
                            :, :, lo:TCH]
                        nc.scalar.activation(p_v, s_v, AF.Exp, scale=0.125)
                    # prefetch two score tiles ahead, across the phase
                    # boundary so the exp stream never refills cold
                    if kc + 2 < nkc:
                        emit_s01(qc, kc + 2)
                    elif kc + 2 - nkc < min(2, npair):
                        emit_s2(qc, kc + 2 - nkc)
                    if m >= 0:
                        nc.vector.tensor_mul(PT[:, ds(lo, 128)],
                                             PT[:, ds(lo, 128)], tri_sb[:])
                        nc.vector.tensor_mul(PT[:, ds(TCH + lo, 128)],
                                             PT[:, ds(TCH + lo, 128)],
                                             tri_sb[:])
                    nc.tensor.matmul(oT0[:, lo:TCH], Vp[:, kc, 0:128],
                                     PT[:, lo:TCH],
                                     start=(kc == 0), stop=(kc == nkc - 1))
                    nc.tensor.matmul(oT1[:, lo:TCH], Vp[:, kc, 128:256],
                                     PT[:, TCH + lo:1024],
                                     start=(kc == 0), stop=(kc == nkc - 1))
                normalize(oT0, 0, qc)
                normalize(oT1, 1, qc)

                # ---- pass 2: head 2, k-chunk pairs row-tiled ----
                oT2 = ops.tile([128, TCH], f32, tag="oT", name="oT2")
                for idx in (0, 1):
                    if (qc, "p2", idx) not in pend and idx < npair:
                        emit_s2(qc, idx)
                for kp in range(npair):
                    if fillers:
                        fillers.pop(0)()
                    kc0, kc1 = 2 * kp, 2 * kp + 1
                    lo0, m0 = lo_of(kc0)
                    lo1, m1 = lo_of(kc1)
                    S2 = pend.pop((qc, "p2", kp))
                    PT2 = ptp.tile([128, 1024], bf16, tag="PT", name="PT2")
                    if lo0 == 0:
                        nc.scalar.activation(PT2[:], S2[:], AF.Exp,
                                             scale=0.125)
                    else:
                        s_v = S2[:].rearrange("p (h q) -> p h q", h=2)[
                            :, :, lo0:TCH]
                        p_v = PT2[:].rearrange("p (h q) -> p h q", h=2)[
                            :, :, lo0:TCH]
                        nc.scalar.activation(p_v, s_v, AF.Exp, scale=0.125)
                    if kp + 2 < npair:
                        emit_s2(qc, kp + 2)
                    elif qc >= 1 and qc + 1 < nt and kp + 2 - npair < 2:
                        # cross into the next q-chunk (its Q/K fillers are
                        # guaranteed emitted by now for qc >= 1)
                        emit_s01(qc + 1, kp + 2 - npair)
                    if m0 >= 0:
                        nc.vector.tensor_mul(PT2[:, ds(lo0, 128)],
                                             PT2[:, ds(lo0, 128)], tri_sb[:])
                    if m1 >= 0:
                        nc.vector.tensor_mul(PT2[:, ds(TCH + lo1, 128)],
                                             PT2[:, ds(TCH + lo1, 128)],
                                             tri_sb[:])
                    nc.tensor.matmul(oT2[:, lo0:TCH], Vp[:, kc0, 256:384],
                                     PT2[:, lo0:TCH],
                                     start=(kp == 0), stop=False)
                    nc.tensor.matmul(oT2[:, lo1:TCH], Vp[:, kc1, 256:384],
                                     PT2[:, TCH + lo1:1024],
                                     start=False, stop=(kp == npair - 1))
                normalize(oT2, 2, qc)

            ops0 = qkv_ops(0)
            emit_weight_loads_head()
            ops0[0]()
            emit_bias_load()
            emit_warm(10)
            emit_weight_loads_rest()
            ops0[1]()
            ops0[2]()
            ops0[3]()
            for qc in range(nt):
                fillers = []
                if qc == 0:
                    fillers += ops0[4:]
                if qc + 1 < nt:
                    fillers += qkv_ops(qc + 1)
                if qc >= 1:
                    fillers += proj_ops(qc - 1)
                attention(qc, fillers)
                for op in fillers:
                    op()
            emit_warm(10)
            for op in proj_ops(nt - 1, tail=True):
                op()

    return nc


def arrange_x(xb):
    import ml_dtypes
    t = xb.shape[0]
    xt = xb.T.reshape(C // CPART, CPART, t // TCH, TCH)
    xt = xt.transpose(1, 2, 0, 3).reshape(CPART, -1)
    return np.ascontiguousarray(xt).astype(ml_dtypes.bfloat16)


def make_tri():
    import ml_dtypes
    p = np.arange(128)[:, None]
    j = np.arange(128)[None, :]
    return (j - p >= 0).astype(ml_dtypes.bfloat16)


def core_inputs(c, x, w_attn, b_attn, w_proj, xT_by_batch, tri):
    import ml_dtypes
    f32 = np.float32
    b = c // 4
    heads = [(c % 4) * HPC + i for i in range(HPC)]
    h0, h1, h2 = heads

    def Wq(h):
        return w_attn[:, h * HD:(h + 1) * HD]

    def Wk(h):
        return w_attn[:, C + h * HD:C + (h + 1) * HD]

    def Wv(h):
        return w_attn[:, 2 * C + h * HD:2 * C + (h + 1) * HD]

    def bq(h):
        return b_attn[h * HD:(h + 1) * HD]

    def bk(h):
        return b_attn[C + h * HD:C + (h + 1) * HD]

    wv192 = np.zeros((C, 192), f32)
    for i, h in enumerate(heads):
        wv192[:, i * 64:(i + 1) * 64] = Wv(h)
    bf = ml_dtypes.bfloat16

    def arr(w):
        m = w.shape[1]
        return np.ascontiguousarray(
            w.reshape(C // CPART, CPART, m).transpose(1, 0, 2).reshape(
                CPART, -1)).astype(bf)

    wp192 = np.concatenate([w_proj[h * HD:(h + 1) * HD, :] for h in heads], 0)
    wpB = np.zeros((CPART, C), np.float32)
    wpB[0:64, :] = wp192[128:192, :]
    wpB[64:128, :] = wp192[128:192, :]
    misc = np.zeros((CPART, 384), np.float32)
    wpk = np.concatenate([
        arr(np.concatenate([Wq(h0), Wq(h1)], 1)).astype(np.float32),
        arr(np.concatenate([Wk(h0), Wk(h1)], 1)).astype(np.float32),
        arr(np.concatenate([Wq(h2), Wk(h2)], 1)).astype(np.float32),
        arr(wv192).astype(np.float32),
        wp192[0:128, :], wpB, tri.astype(np.float32), misc,
    ], axis=1).astype(bf)
    bpk = np.stack([
        np.concatenate([bq(h0), bq(h1)]),
        np.concatenate([bk(h0), bk(h1)]),
        np.concatenate([bq(h2), bk(h2)]),
    ], axis=1).astype(np.float32)
    return {
        "xT": xT_by_batch[b],
        "wpk": np.ascontiguousarray(wpk),
        "bpk": np.ascontiguousarray(bpk),
    }


TRACE = False
LAST_EXEC_NS = None
LAST_RESULTS = None


def kernel(x, w_attn, b_attn, w_proj, b_proj):
    global LAST_EXEC_NS, LAST_RESULTS
    _ensure_axon_hooks_module()
    from concourse.bass_utils import run_bass_kernel_spmd

    x = np.asarray(x, np.float32)
    w_attn = np.asarray(w_attn, np.float32)
    b_attn = np.asarray(b_attn, np.float32)
    w_proj = np.asarray(w_proj, np.float32)
    b_proj = np.asarray(b_proj, np.float32)

    if "nc" not in _cache:
        nc = build_program()
        nc.finalize()
        _cache["nc"] = nc
    nc = _cache["nc"]

    import ml_dtypes  # noqa: F401
    xT_by_batch = [arrange_x(x[b]) for b in range(B)]
    tri = make_tri()
    in_maps = [
        core_inputs(c, x, w_attn, b_attn, w_proj, xT_by_batch, tri)
        for c in range(N_CORES)
    ]
    res = run_bass_kernel_spmd(nc, in_maps, core_ids=list(range(N_CORES)),
                               trace=TRACE)
    LAST_EXEC_NS = res.exec_time_ns
    LAST_RESULTS = res
    out = np.zeros((B, T, C), np.float32)
    for c in range(N_CORES):
        out[c // 4] += np.asarray(res.results[c]["out"], np.float32)
    bv_all = b_attn[2 * C:]
    out += (b_proj + bv_all @ w_proj)[None, None, :]
    return out


# revision 20
# speedup vs baseline: 1.1358x; 1.1358x over previous
"""Causal self-attention on 8 TRN2 NeuronCores.  Baseline schedule +
replicated-denominator normalize (ones-block in V' fills PV rows 64:127
with the softmax denominator, so normalize is a PSUM->SBUF copy + approx
reciprocal + multiply; no GpSimd partition_broadcast on the critical path).

Sharding: core c handles batch b = c//4 and heads [3g, 3g+1, 3g+2] with
g = c%4 (data parallel on B x tensor parallel on heads).  Each core computes
its heads' attention output projected through its slice of w_proj; the host
sums the 4 partial [T, C] outputs per batch and adds b_proj.
"""

import numpy as np

N_CORES = 8
B = 2
T = 4096
C = 768
NH = 12
HD = 64
HPC = 3            # heads per core
TCH = 512          # t / q chunk
KCH = 128          # k chunk
CPART = 128

_cache = {}


def _ensure_axon_hooks_module():
    import sys
    import types
    try:
        import antenv.axon_hooks  # noqa: F401
        return
    except Exception:
        pass
    m = types.ModuleType("antenv.axon_hooks")
    m._hook = None

    def _set(h):
        m._hook = h

    def _get():
        return m._hook

    m.set_axon_ntff_profile_hook = _set
    m.get_axon_ntff_profile_hook = _get
    sys.modules["antenv.axon_hooks"] = m


def build_program(t=T):
    """Build the single-core SPMD bass program (same program on all cores,
    per-core data). Returns the un-finalized Bacc."""
    import concourse.mybir as mybir
    import concourse.tile as tile
    from concourse import bacc
    from concourse.bass import ds, ts

    f32 = mybir.dt.float32
    bf16 = mybir.dt.bfloat16
    AF = mybir.ActivationFunctionType

    nt = t // TCH          # number of t/q chunks
    spk = TCH // KCH       # k-chunks per t-chunk (4)
    cc_n = C // CPART      # 6 contraction chunks

    nc = bacc.Bacc("TRN2", target_bir_lowering=False)

    PK_W = 6 * 128 * 3 + 6 * 192 + C + C + 128 + 384
    xT = nc.dram_tensor("xT", [128, (t // TCH) * (C // CPART) * TCH], bf16,
                        kind="ExternalInput")
    wpk_d = nc.dram_tensor("wpk", [128, PK_W], bf16, kind="ExternalInput")
    bpk_d = nc.dram_tensor("bpk", [128, 3], f32, kind="ExternalInput")
    out_d = nc.dram_tensor("out", [t, C], bf16, kind="ExternalOutput")

    with tile.TileContext(nc) as tc_:
        with (
            tc_.tile_pool(name="consts", bufs=1) as consts,
            tc_.tile_pool(name="big", bufs=1) as big,
            tc_.tile_pool(name="xin", bufs=3) as xin,
            tc_.tile_pool(name="ptp", bufs=10) as ptp,
            tc_.tile_pool(name="wkp", bufs=4) as wkp,
            tc_.tile_pool(name="sps", bufs=2, space="PSUM") as sps,
            tc_.tile_pool(name="ops", bufs=4, space="PSUM") as ops,
        ):
            wpk = consts.tile([128, PK_W], bf16)
            bpk = consts.tile([128, 3], f32)

            def emit_weight_loads_head():
                nc.sync.dma_start(wpk[:, 0:768], wpk_d[:, 0:768])

            def emit_bias_load():
                nc.sync.dma_start(bpk[:], bpk_d[:, :])

            def emit_weight_loads_rest():
                nc.sync.dma_start(wpk[:, 768:2304], wpk_d[:, 768:2304])
                nc.sync.dma_start(wpk[:, 2304:3474], wpk_d[:, 2304:3474])
                nc.sync.dma_start(wpk[:, 3474:PK_W], wpk_d[:, 3474:PK_W])

            warm_src = consts.tile([128, 512], bf16)
            nc.vector.memset(warm_src[:], 0.0)

            def emit_warm(n):
                wps = sps.tile([128, 512], f32, tag="S", name="warm")
                for _ in range(n):
                    nc.tensor.matmul(wps[:], warm_src[:, 0:128],
                                     warm_src[:], start=True, stop=True)

            def seg(off, w):
                ap = wpk[:, off:off + w]
                return ap, off + w

            _o = 0
            wq01_f, _o = seg(_o, 6 * 128)
            wk01_f, _o = seg(_o, 6 * 128)
            wqk2_f, _o = seg(_o, 6 * 128)
            wv_f, _o = seg(_o, 6 * 192)
            wpA_sb, _o = seg(_o, C)
            wpB_full, _o = seg(_o, C)
            tri_sb, _o = seg(_o, 128)
            misc_f, _o = seg(_o, 384)
            wq01_sb = wq01_f.rearrange("p (c m) -> p c m", c=cc_n)
            wk01_sb = wk01_f.rearrange("p (c m) -> p c m", c=cc_n)
            wqk2_sb = wqk2_f.rearrange("p (c m) -> p c m", c=cc_n)
            wv_sb = wv_f.rearrange("p (c m) -> p c m", c=cc_n)
            wpB_lo = wpB_full[0:64, :]
            wpB_hi = wpB_full[64:128, :]
            del misc_f  # reserved pack space, currently unused
            bq01_sb = bpk[:, 0:1]
            bk01_sb = bpk[:, 1:2]
            bqk2_sb = bpk[:, 2:3]

            # ---- persistent activations ----
            Q01 = big.tile([128, t], bf16)
            K01 = big.tile([128, t], bf16)
            Q2 = big.tile([128, t], bf16)
            K2 = big.tile([128, t], bf16)
            Vp = big.tile([128, t // KCH, 384], bf16)
            Vp_h = Vp[:, :, :].rearrange("p n (h x) -> p n h x", h=3)
            yTa = big.tile([128, t], bf16)
            yT2 = big.tile([128, t], bf16)

            xT_r = xT[:, :].rearrange("p (nt c m) -> p nt c m", nt=nt,
                                      c=cc_n)

            def qkv_ops(tci):
                state = {}
                ops_l = []

                def dma_cast():
                    xtb = xin.tile([128, cc_n, TCH], bf16, tag="xtb",
                                   name="xtb")
                    if tci == 0:
                        for cc in range(0, cc_n, 2):
                            nc.sync.dma_start(xtb[:, cc:cc + 2, :],
                                              xT_r[:, tci, cc:cc + 2, :])
                    else:
                        nc.sync.dma_start(xtb[:], xT_r[:, tci, :, :])
                    state["xtb"] = xtb
                ops_l.append(dma_cast)

                def qk_set(wsb, bsb, dst):
                    xtb = state["xtb"]
                    qkps = sps.tile([128, TCH], f32, tag="S", name="qkps")
                    for cc in range(cc_n):
                        nc.tensor.matmul(
                            qkps[:], wsb[:, cc, :], xtb[:, cc, :],
                            start=(cc == 0), stop=(cc == cc_n - 1))
                    if dst is None:
                        nc.vector.tensor_scalar_add(
                            Q2[0:64, ts(tci, TCH)], qkps[0:64, :],
                            bsb[0:64, :])
                        nc.vector.tensor_scalar_add(
                            K2[64:128, ts(tci, TCH)], qkps[64:128, :],
                            bsb[64:128, :])
                        nc.sync.dma_start(Q2[64:128, ts(tci, TCH)],
                                          Q2[0:64, ts(tci, TCH)])
                        nc.sync.dma_start(K2[0:64, ts(tci, TCH)],
                                          K2[64:128, ts(tci, TCH)])
                    else:
                        nc.vector.tensor_scalar_add(
                            dst[:, ts(tci, TCH)], qkps[:], bsb[:])

                for wsb, bsb, dst in (
                    (wq01_sb, bq01_sb, Q01),
                    (wk01_sb, bk01_sb, K01),
                    (wqk2_sb, bqk2_sb, None),
                ):
                    ops_l.append(
                        lambda w=wsb, b=bsb, d=dst: qk_set(w, b, d))

                def v_set(st):
                    xtb = state["xtb"]
                    tt = tci * spk + st
                    vps = ops.tile([128, 192], f32, tag="oT", name="vps")
                    for cc in range(cc_n):
                        nc.tensor.matmul(
                            vps[:], xtb[:, cc, ts(st, 128)], wv_sb[:, cc, :],
                            start=(cc == 0), stop=(cc == cc_n - 1))
                    nc.vector.tensor_copy(
                        Vp_h[:, tt, :, 0:64],
                        vps[:].rearrange("p (h x) -> p h x", h=3))
                    nc.vector.memset(Vp_h[:, tt, :, 64:128], 1.0)

                for st in range(spk):
                    ops_l.append(lambda s=st: v_set(s))
                return ops_l

            def proj_ops(tci, tail=False):
                def do_tile(tt):
                    po1 = ops.tile([128, 512], f32, tag="oT", name="po1")
                    po2 = ops.tile([128, 256], f32, tag="oT", name="po2")
                    for po, cs, cw in ((po1, 0, 512), (po2, 512, 256)):
                        nc.tensor.matmul(po[:], yTa[:, ts(tt, 128)],
                                         wpA_sb[:, ds(cs, cw)],
                                         start=True, stop=False)
                        nc.tensor.matmul(po[:], yT2[0:64, ts(tt, 128)],
                                         wpB_lo[:, ds(cs, cw)],
                                         start=False, stop=True)
                    pout = xin.tile([128, C], bf16, tag="pout", name="pout")
                    if tail:
                        nc.scalar.activation(pout[:, 0:512], po1[:], AF.Copy)
                    else:
                        nc.vector.tensor_copy(pout[:, 0:512], po1[:])
                    nc.vector.tensor_copy(pout[:, 512:768], po2[:])
                    eng = nc.gpsimd if tail else nc.sync
                    eng.dma_start(out_d[ds(tt * 128, 64), :],
                                  pout[0:64, :])
                    eng.dma_start(out_d[ds(tt * 128 + 64, 64), :],
                                  pout[64:128, :])

                return [lambda x=(tci * spk + s): do_tile(x)
                        for s in range(spk)]

            def normalize(oT, h, qc):
                den = wkp.tile([64, TCH], f32, tag="den", name="den")
                nc.vector.tensor_copy(den[:], oT[64:128, :])
                rb = wkp.tile([64, TCH], f32, tag="rb", name="rb")
                nc.vector.reciprocal_approx_fast(out=rb[:], in_=den[:])
                if h == 0:
                    nc.vector.tensor_mul(yTa[0:64, ts(qc, TCH)], oT[0:64, :],
                                         rb[:])
                elif h == 2:
                    nc.vector.tensor_mul(yT2[0:64, ts(qc, TCH)], oT[0:64, :],
                                         rb[:])
                else:
                    y1t = wkp.tile([64, TCH], bf16, tag="y1t", name="y1t")
                    nc.vector.tensor_mul(y1t[:], oT[0:64, :], rb[:])
                    nc.sync.dma_start(yTa[64:128, ts(qc, TCH)], y1t[:])

            spend = {}

            def emit_s01(qc, kc):
                q0 = qc * TCH
                lo = max(0, 128 * (kc - qc * spk))
                S = sps.tile([128, 1024], f32, tag="S", name="S01")
                nc.tensor.matmul(
                    S[:, lo:TCH],
                    K01[0:64, ts(kc, KCH)], Q01[0:64, ds(q0 + lo,
                                                         TCH - lo)],
                    start=True, stop=True, tile_position=(0, 0))
                nc.tensor.matmul(
                    S[:, TCH + lo:1024],
                    K01[64:128, ts(kc, KCH)], Q01[64:128, ds(q0 + lo,
                                                             TCH - lo)],
                    start=True, stop=True, tile_position=(64, 0))
                spend[(qc, "p1", kc)] = S

            def emit_s2(qc, kp):
                q0 = qc * TCH
                kc0, kc1 = 2 * kp, 2 * kp + 1
                lo0 = max(0, 128 * (kc0 - qc * spk))
                S2 = sps.tile([128, 1024], f32, tag="S", name="S2")
                nc.tensor.matmul(
                    S2[:, lo0:TCH],
                    K2[0:64, ts(kc0, KCH)], Q2[0:64, ds(q0 + lo0,
                                                        TCH - lo0)],
                    start=True, stop=True, tile_position=(0, 0))
                nc.tensor.matmul(
                    S2[:, TCH + lo0:1024],
                    K2[64:128, ts(kc1, KCH)], Q2[64:128, ds(q0 + lo0,
                                                            TCH - lo0)],
                    start=True, stop=True, tile_position=(64, 0))
                spend[(qc, "p2", kp)] = S2

            def attention(qc, fillers):
                nkc = (qc + 1) * spk
                npair = nkc // 2
                npop = 2 if qc == 0 else 1

                def lo_of(kc):
                    m = kc - qc * spk
                    return max(0, 128 * m), m

                # ---- pass 1: heads 0,1 row-tiled ----
                oT0 = ops.tile([128, TCH], f32, tag="oT", name="oT0")
                oT1 = ops.tile([128, TCH], f32, tag="oT", name="oT1")
                for idx in (0, 1):
                    if (qc, "p1", idx) not in spend:
                        emit_s01(qc, idx)
                for kc in range(nkc):
                    for _ in range(npop):
                        if fillers:
                            fillers.pop(0)()
                    lo, m = lo_of(kc)
                    S = spend.pop((qc, "p1", kc))
                    PT = ptp.tile([128, 1024], bf16, tag="PT", name="PT")
                    if lo == 0:
                        nc.scalar.activation(PT[:], S[:], AF.Exp, scale=0.125)
                    else:
                        s_v = S[:].rearrange("p (h q) -> p h q", h=2)[
                            :, :, lo:TCH]
                        p_v = PT[:].rearrange("p (h q) -> p h q", h=2)[
                            :, :, lo:TCH]
                        nc.scalar.activation(p_v, s_v, AF.Exp, scale=0.125)
                    if kc + 2 < nkc:
                        emit_s01(qc, kc + 2)
                    elif kc == nkc - 1:
                        # exactly one pair across the phase boundary: its
                        # PSUM slot is already free (exp kc-1 done), so it
                        # does not add to the PE's blocked-instruction park
                        emit_s2(qc, 0)
                    if m >= 0:
                        nc.vector.tensor_mul(PT[:, ds(lo, 128)],
                                             PT[:, ds(lo, 128)], tri_sb[:])
                        nc.vector.tensor_mul(PT[:, ds(TCH + lo, 128)],
                                             PT[:, ds(TCH + lo, 128)],
                                             tri_sb[:])
                    nc.tensor.matmul(oT0[:, lo:TCH], Vp[:, kc, 0:128],
                                     PT[:, lo:TCH],
                                     start=(kc == 0), stop=(kc == nkc - 1))
                    nc.tensor.matmul(oT1[:, lo:TCH], Vp[:, kc, 128:256],
                                     PT[:, TCH + lo:1024],
                                     start=(kc == 0), stop=(kc == nkc - 1))
                normalize(oT0, 0, qc)
                normalize(oT1, 1, qc)

                # ---- pass 2: head 2, k-chunk pairs row-tiled ----
                oT2 = ops.tile([128, TCH], f32, tag="oT", name="oT2")
                for idx in (0, 1):
                    if (qc, "p2", idx) not in spend and idx < npair:
                        emit_s2(qc, idx)
                for kp in range(npair):
                    for _ in range(npop):
                        if fillers:
                            fillers.pop(0)()
                    kc0, kc1 = 2 * kp, 2 * kp + 1
                    lo0, m0 = lo_of(kc0)
                    lo1, m1 = lo_of(kc1)
                    S2 = spend.pop((qc, "p2", kp))
                    PT2 = ptp.tile([128, 1024], bf16, tag="PT", name="PT2")
                    if lo0 == 0:
                        nc.scalar.activation(PT2[:], S2[:], AF.Exp,
                                             scale=0.125)
                    else:
                        s_v = S2[:].rearrange("p (h q) -> p h q", h=2)[
                            :, :, lo0:TCH]
                        p_v = PT2[:].rearrange("p (h q) -> p h q", h=2)[
                            :, :, lo0:TCH]
                        nc.scalar.activation(p_v, s_v, AF.Exp, scale=0.125)
                    if kp + 2 < npair:
                        emit_s2(qc, kp + 2)
                    elif kp == npair - 1 and qc + 1 < nt:
                        # one pair across the q-chunk boundary (Q01/K01 of
                        # qc+1 were emitted as fillers early in this qc)
                        emit_s01(qc + 1, 0)
                    if m0 >= 0:
                        nc.vector.tensor_mul(PT2[:, ds(lo0, 128)],
                                             PT2[:, ds(lo0, 128)], tri_sb[:])
                    if m1 >= 0:
                        nc.vector.tensor_mul(PT2[:, ds(TCH + lo1, 128)],
                                             PT2[:, ds(TCH + lo1, 128)],
                                             tri_sb[:])
                    nc.tensor.matmul(oT2[:, lo0:TCH], Vp[:, kc0, 256:384],
                                     PT2[:, lo0:TCH],
                                     start=(kp == 0), stop=False)
                    nc.tensor.matmul(oT2[:, lo1:TCH], Vp[:, kc1, 256:384],
                                     PT2[:, TCH + lo1:1024],
                                     start=False, stop=(kp == npair - 1))
                normalize(oT2, 2, qc)

            ops0 = qkv_ops(0)
            emit_weight_loads_head()
            ops0[0]()
            emit_bias_load()
            emit_warm(10)
            emit_weight_loads_rest()
            ops0[1]()
            ops0[2]()
            ops0[3]()
            for qc in range(nt):
                fillers = []
                if qc == 0:
                    fillers += ops0[4:]
                if qc + 1 < nt:
                    fillers += qkv_ops(qc + 1)
                if qc >= 1:
                    fillers += proj_ops(qc - 1)
                attention(qc, fillers)
                for op in fillers:
                    op()
            emit_warm(4)
            for op in proj_ops(nt - 1, tail=True):
                op()

    return nc


def arrange_x(xb):
    import ml_dtypes
    t = xb.shape[0]
    xt = xb.T.reshape(C // CPART, CPART, t // TCH, TCH)
    xt = xt.transpose(1, 2, 0, 3).reshape(CPART, -1)
    return np.ascontiguousarray(xt).astype(ml_dtypes.bfloat16)


def make_tri():
    import ml_dtypes
    p = np.arange(128)[:, None]
    j = np.arange(128)[None, :]
    return (j - p >= 0).astype(ml_dtypes.bfloat16)


def core_inputs(c, x, w_attn, b_attn, w_proj, xT_by_batch, tri):
    import ml_dtypes
    f32 = np.float32
    b = c // 4
    heads = [(c % 4) * HPC + i for i in range(HPC)]
    h0, h1, h2 = heads

    def Wq(h):
        return w_attn[:, h * HD:(h + 1) * HD]

    def Wk(h):
        return w_attn[:, C + h * HD:C + (h + 1) * HD]

    def Wv(h):
        return w_attn[:, 2 * C + h * HD:2 * C + (h + 1) * HD]

    def bq(h):
        return b_attn[h * HD:(h + 1) * HD]

    def bk(h):
        return b_attn[C + h * HD:C + (h + 1) * HD]

    wv192 = np.zeros((C, 192), f32)
    for i, h in enumerate(heads):
        wv192[:, i * 64:(i + 1) * 64] = Wv(h)
    bf = ml_dtypes.bfloat16

    def arr(w):
        m = w.shape[1]
        return np.ascontiguousarray(
            w.reshape(C // CPART, CPART, m).transpose(1, 0, 2).reshape(
                CPART, -1)).astype(bf)

    wp192 = np.concatenate([w_proj[h * HD:(h + 1) * HD, :] for h in heads], 0)
    wpB = np.zeros((CPART, C), np.float32)
    wpB[0:64, :] = wp192[128:192, :]
    wpB[64:128, :] = wp192[128:192, :]
    misc = np.zeros((CPART, 384), np.float32)
    wpk = np.concatenate([
        arr(np.concatenate([Wq(h0), Wq(h1)], 1)).astype(np.float32),
        arr(np.concatenate([Wk(h0), Wk(h1)], 1)).astype(np.float32),
        arr(np.concatenate([Wq(h2), Wk(h2)], 1)).astype(np.float32),
        arr(wv192).astype(np.float32),
        wp192[0:128, :], wpB, tri.astype(np.float32), misc,
    ], axis=1).astype(bf)
    bpk = np.stack([
        np.concatenate([bq(h0), bq(h1)]),
        np.concatenate([bk(h0), bk(h1)]),
        np.concatenate([bq(h2), bk(h2)]),
    ], axis=1).astype(np.float32)
    return {
        "xT": xT_by_batch[b],
        "wpk": np.ascontiguousarray(wpk),
        "bpk": np.ascontiguousarray(bpk),
    }


TRACE = False
LAST_EXEC_NS = None
LAST_RESULTS = None


def kernel(x, w_attn, b_attn, w_proj, b_proj):
    global LAST_EXEC_NS, LAST_RESULTS
    _ensure_axon_hooks_module()
    from concourse.bass_utils import run_bass_kernel_spmd

    x = np.asarray(x, np.float32)
    w_attn = np.asarray(w_attn, np.float32)
    b_attn = np.asarray(b_attn, np.float32)
    w_proj = np.asarray(w_proj, np.float32)
    b_proj = np.asarray(b_proj, np.float32)

    if "nc" not in _cache:
        nc = build_program()
        nc.finalize()
        _cache["nc"] = nc
    nc = _cache["nc"]

    import ml_dtypes  # noqa: F401
    xT_by_batch = [arrange_x(x[b]) for b in range(B)]
    tri = make_tri()
    in_maps = [
        core_inputs(c, x, w_attn, b_attn, w_proj, xT_by_batch, tri)
        for c in range(N_CORES)
    ]
    res = run_bass_kernel_spmd(nc, in_maps, core_ids=list(range(N_CORES)),
                               trace=TRACE)
    LAST_EXEC_NS = res.exec_time_ns
    LAST_RESULTS = res
    out = np.zeros((B, T, C), np.float32)
    for c in range(N_CORES):
        out[c // 4] += np.asarray(res.results[c]["out"], np.float32)
    bv_all = b_attn[2 * C:]
    out += (b_proj + bv_all @ w_proj)[None, None, :]
    return out


# revision 22
# speedup vs baseline: 1.1779x; 1.0371x over previous
"""Causal self-attention on 8 TRN2 NeuronCores.  Baseline schedule +
replicated-denominator normalize (ones-block in V' fills PV rows 64:127
with the softmax denominator, so normalize is a PSUM->SBUF copy + approx
reciprocal + multiply; no GpSimd partition_broadcast on the critical path).

Sharding: core c handles batch b = c//4 and heads [3g, 3g+1, 3g+2] with
g = c%4 (data parallel on B x tensor parallel on heads).  Each core computes
its heads' attention output projected through its slice of w_proj; the host
sums the 4 partial [T, C] outputs per batch and adds b_proj.
"""

import numpy as np

N_CORES = 8
B = 2
T = 4096
C = 768
NH = 12
HD = 64
HPC = 3            # heads per core
TCH = 512          # t / q chunk
KCH = 128          # k chunk
CPART = 128

_cache = {}


def _ensure_axon_hooks_module():
    import sys
    import types
    try:
        import antenv.axon_hooks  # noqa: F401
        return
    except Exception:
        pass
    m = types.ModuleType("antenv.axon_hooks")
    m._hook = None

    def _set(h):
        m._hook = h

    def _get():
        return m._hook

    m.set_axon_ntff_profile_hook = _set
    m.get_axon_ntff_profile_hook = _get
    sys.modules["antenv.axon_hooks"] = m


def build_program(t=T):
    """Build the single-core SPMD bass program (same program on all cores,
    per-core data). Returns the un-finalized Bacc."""
    import concourse.mybir as mybir
    import concourse.tile as tile
    from concourse import bacc
    from concourse.bass import ds, ts

    f32 = mybir.dt.float32
    bf16 = mybir.dt.bfloat16
    AF = mybir.ActivationFunctionType

    nt = t // TCH          # number of t/q chunks
    spk = TCH // KCH       # k-chunks per t-chunk (4)
    cc_n = C // CPART      # 6 contraction chunks

    nc = bacc.Bacc("TRN2", target_bir_lowering=False)

    PK_W = 6 * 128 * 3 + 6 * 192 + C + C + 128 + 384
    xT = nc.dram_tensor("xT", [128, (t // TCH) * (C // CPART) * TCH], bf16,
                        kind="ExternalInput")
    wpk_d = nc.dram_tensor("wpk", [128, PK_W], bf16, kind="ExternalInput")
    bpk_d = nc.dram_tensor("bpk", [128, 3], f32, kind="ExternalInput")
    out_d = nc.dram_tensor("out", [t, C], bf16, kind="ExternalOutput")

    with tile.TileContext(nc) as tc_:
        with (
            tc_.tile_pool(name="consts", bufs=1) as consts,
            tc_.tile_pool(name="big", bufs=1) as big,
            tc_.tile_pool(name="xin", bufs=3) as xin,
            tc_.tile_pool(name="ptp", bufs=10) as ptp,
            tc_.tile_pool(name="wkp", bufs=4) as wkp,
            tc_.tile_pool(name="sps", bufs=2, space="PSUM") as sps,
            tc_.tile_pool(name="ops", bufs=4, space="PSUM") as ops,
        ):
            wpk = consts.tile([128, PK_W], bf16)
            bpk = consts.tile([128, 3], f32)

            def emit_weight_loads_head():
                nc.sync.dma_start(wpk[:, 0:768], wpk_d[:, 0:768])

            def emit_bias_load():
                nc.sync.dma_start(bpk[:], bpk_d[:, :])

            def emit_weight_loads_rest():
                nc.sync.dma_start(wpk[:, 768:2304], wpk_d[:, 768:2304])
                nc.sync.dma_start(wpk[:, 2304:3474], wpk_d[:, 2304:3474])
                nc.sync.dma_start(wpk[:, 3474:PK_W], wpk_d[:, 3474:PK_W])

            warm_src = consts.tile([128, 512], bf16)
            nc.vector.memset(warm_src[:], 0.0)

            def emit_warm(n):
                wps = sps.tile([128, 512], f32, tag="S", name="warm")
                for _ in range(n):
                    nc.tensor.matmul(wps[:], warm_src[:, 0:128],
                                     warm_src[:], start=True, stop=True)

            def seg(off, w):
                ap = wpk[:, off:off + w]
                return ap, off + w

            _o = 0
            wq01_f, _o = seg(_o, 6 * 128)
            wk01_f, _o = seg(_o, 6 * 128)
            wqk2_f, _o = seg(_o, 6 * 128)
            wv_f, _o = seg(_o, 6 * 192)
            wpA_sb, _o = seg(_o, C)
            wpB_full, _o = seg(_o, C)
            tri_sb, _o = seg(_o, 128)
            misc_f, _o = seg(_o, 384)
            wq01_sb = wq01_f.rearrange("p (c m) -> p c m", c=cc_n)
            wk01_sb = wk01_f.rearrange("p (c m) -> p c m", c=cc_n)
            wqk2_sb = wqk2_f.rearrange("p (c m) -> p c m", c=cc_n)
            wv_sb = wv_f.rearrange("p (c m) -> p c m", c=cc_n)
            wpB_lo = wpB_full[0:64, :]
            wpB_hi = wpB_full[64:128, :]
            del misc_f  # reserved pack space, currently unused
            bq01_sb = bpk[:, 0:1]
            bk01_sb = bpk[:, 1:2]
            bqk2_sb = bpk[:, 2:3]

            # ---- persistent activations ----
            Q01 = big.tile([128, t], bf16)
            K01 = big.tile([128, t], bf16)
            Q2 = big.tile([128, t], bf16)
            K2 = big.tile([128, t], bf16)
            Vp = big.tile([128, t // KCH, 384], bf16)
            Vp_h = Vp[:, :, :].rearrange("p n (h x) -> p n h x", h=3)
            yTa = big.tile([128, t], bf16)
            yT2 = big.tile([128, t], bf16)

            xT_r = xT[:, :].rearrange("p (nt c m) -> p nt c m", nt=nt,
                                      c=cc_n)

            def qkv_ops(tci):
                state = {}
                ops_l = []

                def dma_cast():
                    xtb = xin.tile([128, cc_n, TCH], bf16, tag="xtb",
                                   name="xtb")
                    if tci == 0:
                        for cc in range(0, cc_n, 2):
                            nc.sync.dma_start(xtb[:, cc:cc + 2, :],
                                              xT_r[:, tci, cc:cc + 2, :])
                    else:
                        nc.sync.dma_start(xtb[:], xT_r[:, tci, :, :])
                    state["xtb"] = xtb
                ops_l.append(dma_cast)

                def qk_set(wsb, bsb, dst):
                    xtb = state["xtb"]
                    qkps = sps.tile([128, TCH], f32, tag="S", name="qkps")
                    for cc in range(cc_n):
                        nc.tensor.matmul(
                            qkps[:], wsb[:, cc, :], xtb[:, cc, :],
                            start=(cc == 0), stop=(cc == cc_n - 1))
                    if dst is None:
                        nc.vector.tensor_scalar_add(
                            Q2[0:64, ts(tci, TCH)], qkps[0:64, :],
                            bsb[0:64, :])
                        nc.vector.tensor_scalar_add(
                            K2[64:128, ts(tci, TCH)], qkps[64:128, :],
                            bsb[64:128, :])
                        nc.sync.dma_start(Q2[64:128, ts(tci, TCH)],
                                          Q2[0:64, ts(tci, TCH)])
                        nc.sync.dma_start(K2[0:64, ts(tci, TCH)],
                                          K2[64:128, ts(tci, TCH)])
                    else:
                        nc.vector.tensor_scalar_add(
                            dst[:, ts(tci, TCH)], qkps[:], bsb[:])

                for wsb, bsb, dst in (
                    (wq01_sb, bq01_sb, Q01),
                    (wk01_sb, bk01_sb, K01),
                    (wqk2_sb, bqk2_sb, None),
                ):
                    ops_l.append(
                        lambda w=wsb, b=bsb, d=dst: qk_set(w, b, d))

                def v_set(st):
                    xtb = state["xtb"]
                    tt = tci * spk + st
                    vps = ops.tile([128, 192], f32, tag="oT", name="vps")
                    for cc in range(cc_n):
                        nc.tensor.matmul(
                            vps[:], xtb[:, cc, ts(st, 128)], wv_sb[:, cc, :],
                            start=(cc == 0), stop=(cc == cc_n - 1))
                    nc.vector.tensor_copy(
                        Vp_h[:, tt, :, 0:64],
                        vps[:].rearrange("p (h x) -> p h x", h=3))
                    nc.vector.memset(Vp_h[:, tt, :, 64:128], 1.0)

                for st in range(spk):
                    ops_l.append(lambda s=st: v_set(s))
                return ops_l

            def proj_ops(tci, tail=False):
                def do_tile(tt):
                    po1 = ops.tile([128, 512], f32, tag="oT", name="po1")
                    po2 = ops.tile([128, 256], f32, tag="oT", name="po2")
                    for po, cs, cw in ((po1, 0, 512), (po2, 512, 256)):
                        nc.tensor.matmul(po[:], yTa[:, ts(tt, 128)],
                                         wpA_sb[:, ds(cs, cw)],
                                         start=True, stop=False)
                        nc.tensor.matmul(po[:], yT2[0:64, ts(tt, 128)],
                                         wpB_lo[:, ds(cs, cw)],
                                         start=False, stop=True)
                    pout = xin.tile([128, C], bf16, tag="pout", name="pout")
                    if tail:
                        nc.scalar.activation(pout[:, 0:512], po1[:], AF.Copy)
                    else:
                        nc.vector.tensor_copy(pout[:, 0:512], po1[:])
                    nc.vector.tensor_copy(pout[:, 512:768], po2[:])
                    eng = nc.gpsimd if tail else nc.sync
                    eng.dma_start(out_d[ds(tt * 128, 64), :],
                                  pout[0:64, :])
                    eng.dma_start(out_d[ds(tt * 128 + 64, 64), :],
                                  pout[64:128, :])

                return [lambda x=(tci * spk + s): do_tile(x)
                        for s in range(spk)]

            def normalize(oT, h, qc):
                den = wkp.tile([64, TCH], f32, tag="den", name="den")
                nc.vector.tensor_copy(den[:], oT[64:128, :])
                rb = wkp.tile([64, TCH], f32, tag="rb", name="rb")
                nc.vector.reciprocal_approx_fast(out=rb[:], in_=den[:])
                if h == 0:
                    nc.vector.tensor_mul(yTa[0:64, ts(qc, TCH)], oT[0:64, :],
                                         rb[:])
                elif h == 2:
                    nc.vector.tensor_mul(yT2[0:64, ts(qc, TCH)], oT[0:64, :],
                                         rb[:])
                else:
                    y1t = wkp.tile([64, TCH], bf16, tag="y1t", name="y1t")
                    nc.vector.tensor_mul(y1t[:], oT[0:64, :], rb[:])
                    nc.sync.dma_start(yTa[64:128, ts(qc, TCH)], y1t[:])

            def attention(qc, fillers):
                nkc = (qc + 1) * spk
                q0 = qc * TCH
                npop = 2 if qc == 0 else 1

                def lo_of(kc):
                    m = kc - qc * spk
                    return max(0, 128 * m), m

                oT0 = ops.tile([128, TCH], f32, tag="oT", name="oT0")
                oT1 = ops.tile([128, TCH], f32, tag="oT", name="oT1")
                s_pend = {}

                def emit_s01(kc):
                    lo, _ = lo_of(kc)
                    S = sps.tile([128, 1024], f32, tag="S", name="S01")
                    nc.tensor.matmul(
                        S[:, lo:TCH],
                        K01[0:64, ts(kc, KCH)], Q01[0:64, ds(q0 + lo,
                                                             TCH - lo)],
                        start=True, stop=True, tile_position=(0, 0))
                    nc.tensor.matmul(
                        S[:, TCH + lo:1024],
                        K01[64:128, ts(kc, KCH)], Q01[64:128, ds(q0 + lo,
                                                                 TCH - lo)],
                        start=True, stop=True, tile_position=(64, 0))
                    s_pend[kc] = S

                emit_s01(0)
                if nkc > 1:
                    emit_s01(1)
                for kc in range(nkc):
                    for _ in range(npop):
                        if fillers:
                            fillers.pop(0)()
                    lo, m = lo_of(kc)
                    S = s_pend.pop(kc)
                    PT = ptp.tile([128, 1024], bf16, tag="PT", name="PT")
                    if lo == 0:
                        nc.scalar.activation(PT[:], S[:], AF.Exp, scale=0.125)
                    else:
                        s_v = S[:].rearrange("p (h q) -> p h q", h=2)[
                            :, :, lo:TCH]
                        p_v = PT[:].rearrange("p (h q) -> p h q", h=2)[
                            :, :, lo:TCH]
                        nc.scalar.activation(p_v, s_v, AF.Exp, scale=0.125)
                    if kc + 2 < nkc:
                        emit_s01(kc + 2)
                    if m >= 0:
                        nc.vector.tensor_mul(PT[:, ds(lo, 128)],
                                             PT[:, ds(lo, 128)], tri_sb[:])
                        nc.vector.tensor_mul(PT[:, ds(TCH + lo, 128)],
                                             PT[:, ds(TCH + lo, 128)],
                                             tri_sb[:])
                    nc.tensor.matmul(oT0[:, lo:TCH], Vp[:, kc, 0:128],
                                     PT[:, lo:TCH],
                                     start=(kc == 0), stop=(kc == nkc - 1))
                    nc.tensor.matmul(oT1[:, lo:TCH], Vp[:, kc, 128:256],
                                     PT[:, TCH + lo:1024],
                                     start=(kc == 0), stop=(kc == nkc - 1))
                normalize(oT0, 0, qc)
                normalize(oT1, 1, qc)

                oT2 = ops.tile([128, TCH], f32, tag="oT", name="oT2")
                npair = nkc // 2
                s2_pend = {}

                def emit_s2(kp):
                    kc0, kc1 = 2 * kp, 2 * kp + 1
                    lo0, _ = lo_of(kc0)
                    S2 = sps.tile([128, 1024], f32, tag="S", name="S2")
                    nc.tensor.matmul(
                        S2[:, lo0:TCH],
                        K2[0:64, ts(kc0, KCH)], Q2[0:64, ds(q0 + lo0,
                                                            TCH - lo0)],
                        start=True, stop=True, tile_position=(0, 0))
                    nc.tensor.matmul(
                        S2[:, TCH + lo0:1024],
                        K2[64:128, ts(kc1, KCH)], Q2[64:128, ds(q0 + lo0,
                                                                TCH - lo0)],
                        start=True, stop=True, tile_position=(64, 0))
                    s2_pend[kp] = S2

                emit_s2(0)
                if npair > 1:
                    emit_s2(1)
                for kp in range(npair):
                    for _ in range(npop):
                        if fillers:
                            fillers.pop(0)()
                    kc0, kc1 = 2 * kp, 2 * kp + 1
                    lo0, m0 = lo_of(kc0)
                    lo1, m1 = lo_of(kc1)
                    S2 = s2_pend.pop(kp)
                    PT2 = ptp.tile([128, 1024], bf16, tag="PT", name="PT2")
                    if lo0 == 0:
                        nc.scalar.activation(PT2[:], S2[:], AF.Exp,
                                             scale=0.125)
                    else:
                        s_v = S2[:].rearrange("p (h q) -> p h q", h=2)[
                            :, :, lo0:TCH]
                        p_v = PT2[:].rearrange("p (h q) -> p h q", h=2)[
                            :, :, lo0:TCH]
                        nc.scalar.activation(p_v, s_v, AF.Exp, scale=0.125)
                    if kp + 2 < npair:
                        emit_s2(kp + 2)
                    if m0 >= 0:
                        nc.vector.tensor_mul(PT2[:, ds(lo0, 128)],
                                             PT2[:, ds(lo0, 128)], tri_sb[:])
                    if m1 >= 0:
                        nc.vector.tensor_mul(PT2[:, ds(TCH + lo1, 128)],
                                             PT2[:, ds(TCH + lo1, 128)],
                                             tri_sb[:])
                    nc.tensor.matmul(oT2[:, lo0:TCH], Vp[:, kc0, 256:384],
                                     PT2[:, lo0:TCH],
                                     start=(kp == 0), stop=False)
                    nc.tensor.matmul(oT2[:, lo1:TCH], Vp[:, kc1, 256:384],
                                     PT2[:, TCH + lo1:1024],
                                     start=False, stop=(kp == npair - 1))
                normalize(oT2, 2, qc)

            ops0 = qkv_ops(0)
            emit_weight_loads_head()
            ops0[0]()
            emit_bias_load()
            emit_warm(10)
            emit_weight_loads_rest()
            ops0[1]()
            ops0[2]()
            ops0[3]()
            for qc in range(nt):
                fillers = []
                if qc == 0:
                    fillers += ops0[4:]
                if qc + 1 < nt:
                    fillers += qkv_ops(qc + 1)
                if qc >= 1:
                    fillers += proj_ops(qc - 1)
                attention(qc, fillers)
                for op in fillers:
                    op()
            emit_warm(4)
            for op in proj_ops(nt - 1, tail=True):
                op()

    return nc


def arrange_x(xb):
    import ml_dtypes
    t = xb.shape[0]
    xt = xb.T.reshape(C // CPART, CPART, t // TCH, TCH)
    xt = xt.transpose(1, 2, 0, 3).reshape(CPART, -1)
    return np.ascontiguousarray(xt).astype(ml_dtypes.bfloat16)


def make_tri():
    import ml_dtypes
    p = np.arange(128)[:, None]
    j = np.arange(128)[None, :]
    return (j - p >= 0).astype(ml_dtypes.bfloat16)


def core_inputs(c, x, w_attn, b_attn, w_proj, xT_by_batch, tri):
    import ml_dtypes
    f32 = np.float32
    b = c // 4
    heads = [(c % 4) * HPC + i for i in range(HPC)]
    h0, h1, h2 = heads

    def Wq(h):
        return w_attn[:, h * HD:(h + 1) * HD]

    def Wk(h):
        return w_attn[:, C + h * HD:C + (h + 1) * HD]

    def Wv(h):
        return w_attn[:, 2 * C + h * HD:2 * C + (h + 1) * HD]

    def bq(h):
        return b_attn[h * HD:(h + 1) * HD]

    def bk(h):
        return b_attn[C + h * HD:C + (h + 1) * HD]

    wv192 = np.zeros((C, 192), f32)
    for i, h in enumerate(heads):
        wv192[:, i * 64:(i + 1) * 64] = Wv(h)
    bf = ml_dtypes.bfloat16

    def arr(w):
        m = w.shape[1]
        return np.ascontiguousarray(
            w.reshape(C // CPART, CPART, m).transpose(1, 0, 2).reshape(
                CPART, -1)).astype(bf)

    wp192 = np.concatenate([w_proj[h * HD:(h + 1) * HD, :] for h in heads], 0)
    wpB = np.zeros((CPART, C), np.float32)
    wpB[0:64, :] = wp192[128:192, :]
    wpB[64:128, :] = wp192[128:192, :]
    misc = np.zeros((CPART, 384), np.float32)
    wpk = np.concatenate([
        arr(np.concatenate([Wq(h0), Wq(h1)], 1)).astype(np.float32),
        arr(np.concatenate([Wk(h0), Wk(h1)], 1)).astype(np.float32),
        arr(np.concatenate([Wq(h2), Wk(h2)], 1)).astype(np.float32),
        arr(wv192).astype(np.float32),
        wp192[0:128, :], wpB, tri.astype(np.float32), misc,
    ], axis=1).astype(bf)
    bpk = np.stack([
        np.concatenate([bq(h0), bq(h1)]),
        np.concatenate([bk(h0), bk(h1)]),
        np.concatenate([bq(h2), bk(h2)]),
    ], axis=1).astype(np.float32)
    return {
        "xT": xT_by_batch[b],
        "wpk": np.ascontiguousarray(wpk),
        "bpk": np.ascontiguousarray(bpk),
    }


TRACE = False
LAST_EXEC_NS = None
LAST_RESULTS = None


def kernel(x, w_attn, b_attn, w_proj, b_proj):
    global LAST_EXEC_NS, LAST_RESULTS
    _ensure_axon_hooks_module()
    from concourse.bass_utils import run_bass_kernel_spmd

    x = np.asarray(x, np.float32)
    w_attn = np.asarray(w_attn, np.float32)
    b_attn = np.asarray(b_attn, np.float32)
    w_proj = np.asarray(w_proj, np.float32)
    b_proj = np.asarray(b_proj, np.float32)

    if "nc" not in _cache:
        nc = build_program()
        nc.finalize()
        _cache["nc"] = nc
    nc = _cache["nc"]

    import ml_dtypes  # noqa: F401
    xT_by_batch = [arrange_x(x[b]) for b in range(B)]
    tri = make_tri()
    in_maps = [
        core_inputs(c, x, w_attn, b_attn, w_proj, xT_by_batch, tri)
        for c in range(N_CORES)
    ]
    res = run_bass_kernel_spmd(nc, in_maps, core_ids=list(range(N_CORES)),
                               trace=TRACE)
    LAST_EXEC_NS = res.exec_time_ns
    LAST_RESULTS = res
    out = np.zeros((B, T, C), np.float32)
    for c in range(N_CORES):
        out[c // 4] += np.asarray(res.results[c]["out"], np.float32)
    bv_all = b_attn[2 * C:]
    out += (b_proj + bv_all @ w_proj)[None, None, :]
    return out
